# revision 17
# baseline (speedup 1.0000x reference)
"""DeBERTa-RoPE self-attention on 8 Trainium2 cores.

Sharding: data-parallel over batch (4) x tensor-parallel over heads (2 groups
of 8). Each core computes qkv projection for its (batch, head-group), RoPE,
attention, and a row-parallel partial out-projection. The host sums the two
partials per batch (the TP all-reduce) and assembles the full output.

Device layout is "transposed world": activations live as [dims, seq] so every
matmul contraction sits on the partition axis with no on-device transposes.
Masking is folded into v (and an appended mask column produces the softmax
denominator for free); softmax skips max-subtraction (|scores| <= ~5 here).
"""

import numpy as np

import concourse.bass as bass
import concourse.mybir as mybir
import concourse.tile as tile
from concourse.bass_utils import run_bass_kernel_spmd

H = 16
D = 64
HID = H * D
B = 4
S = 1024
THETA = 10000.0
NCORES = 8
HPC = H // 2          # heads per core
KT = HID // 128       # 8 k-tiles
ST = S // 128         # 8 seq tiles

F32 = mybir.dt.float32
F32R = mybir.dt.float32r
AF = mybir.ActivationFunctionType
ALU = mybir.AluOpType


def _r(ap):
    return ap.bitcast(F32R)


def build_program():
    nc = bass.Bass()
    xT = nc.declare_dram_parameter("xT", [HID, S], F32R, isOutput=False)
    wqk = nc.declare_dram_parameter("wqk", [HID, 1024], F32R, isOutput=False)
    wv = nc.declare_dram_parameter("wv", [HID, 512], F32R, isOutput=False)
    bqk = nc.declare_dram_parameter("bqk", [128, 8], F32, isOutput=False)
    bqksh = nc.declare_dram_parameter("bqksh", [128, 8], F32, isOutput=False)
    cosT = nc.declare_dram_parameter("cosT", [128, S], F32, isOutput=False)
    sinT = nc.declare_dram_parameter("sinT", [128, S], F32, isOutput=False)
    mcol = nc.declare_dram_parameter("mcol", [128, ST], F32, isOutput=False)
    wout = nc.declare_dram_parameter("wout", [512, HID], F32R, isOutput=False)
    permT = nc.declare_dram_parameter("permT", [128, 128], F32R, isOutput=False)
    yT = nc.declare_dram_parameter("yT", [HID, S], F32, isOutput=True)

    with tile.TileContext(nc) as tc:
        with (
            tc.tile_pool(name="const", bufs=1) as cpool,
            tc.tile_pool(name="persist", bufs=1) as persist,
        ):
            cos_sb = cpool.tile([128, S], F32)
            sin_sb = cpool.tile([128, S], F32)
            mcol_sb = cpool.tile([128, ST], F32)
            bqk_sb = cpool.tile([128, 8], F32)
            bqksh_sb = cpool.tile([128, 8], F32)
            permT_sb = cpool.tile([128, 128], F32R)

            rope_sb = persist.tile([128, 8, S], F32R)
            vmask_sb = persist.tile([128, ST, HPC * 65], F32R)
            ctxn_sb = persist.tile([128, 4, S], F32R)

            # ---------------- Phase A: projections + RoPE + v ----------------
            with tc.tile_pool(name="phA", bufs=1) as pa:
                xT_sb = pa.tile([128, KT, S], F32R)
                wqk_sb = pa.tile([128, KT, 1024], F32R)
                wv_sb = pa.tile([128, KT, 512], F32R)
                for kt in range(KT):
                    nc.sync.dma_start(
                        xT_sb[:, kt, :], xT[kt * 128:(kt + 1) * 128, :])
                    nc.gpsimd.dma_start(
                        wv_sb[:, kt, :], wv[kt * 128:(kt + 1) * 128, :])
                nc.gpsimd.dma_start(mcol_sb[:], mcol[:])
                nc.gpsimd.dma_start(bqk_sb[:], bqk[:])
                nc.gpsimd.dma_start(bqksh_sb[:], bqksh[:])
                nc.gpsimd.dma_start(permT_sb[:], permT[:])
                nc.gpsimd.dma_start(cos_sb[:], cosT[:])
                nc.gpsimd.dma_start(sin_sb[:], sinT[:])
                for kt in range(KT):
                    nc.scalar.dma_start(
                        wqk_sb[:, kt, :], wqk[kt * 128:(kt + 1) * 128, :])

                # v natural [t, d]: kt-outer so MMs stream behind the DMAs
                with tc.tile_pool(name="psV", bufs=1, space="PSUM") as psV:
                    vps = [psV.tile([128, 512], F32, tag=f"v{tt}",
                                    name=f"vps{tt}")
                           for tt in range(ST)]
                    for kt in range(KT):
                        for tt in range(ST):
                            nc.tensor.matmul(
                                vps[tt][:],
                                _r(xT_sb[:, kt, tt * 128:(tt + 1) * 128]),
                                _r(wv_sb[:, kt, :]),
                                start=(kt == 0), stop=(kt == KT - 1),
                            )
                    for tt in range(ST):
                        vv = vmask_sb[:, tt, :].rearrange(
                            "p (h j) -> p h j", j=65)
                        nc.scalar.activation(
                            vv[:, :, 0:64],
                            vps[tt][:].rearrange("p (h d) -> p h d", d=64),
                            AF.Copy, scale=mcol_sb[:, tt:tt + 1])
                        nc.gpsimd.tensor_copy(
                            vv[:, :, 64:65],
                            mcol_sb[:, tt:tt + 1].broadcast_to([128, HPC, 1]))

                # qkT in (q-pair, k-pair) chunks; kt-outer inside each chunk
                with (
                    tc.tile_pool(name="qksb", bufs=3) as qkp,
                    tc.tile_pool(name="ropetmp", bufs=3) as rt,
                    tc.tile_pool(name="psA", bufs=1, space="PSUM") as psA,
                    tc.tile_pool(name="psSh", bufs=2, space="PSUM") as psSh,
                ):
                    for p in range(4):
                        ms = (p, p + 4)
                        pss = {m: psA.tile([128, S], F32, tag=f"qk{m // 4}",
                                           name=f"psqk{m}")
                               for m in ms}
                        for kt in range(KT):
                            for m in ms:
                                for ch in range(2):
                                    nc.tensor.matmul(
                                        pss[m][:, ch * 512:(ch + 1) * 512],
                                        _r(wqk_sb[:, kt, m * 128:(m + 1) * 128]),
                                        _r(xT_sb[:, kt, ch * 512:(ch + 1) * 512]),
                                        start=(kt == 0), stop=(kt == KT - 1),
                                    )
                        for m in ms:
                            ps_qk = pss[m]
                            qk_sb = qkp.tile([128, S], F32R, tag="qksb")
                            nc.scalar.copy(qk_sb[:], ps_qk[:])
                            ps_sh = psSh.tile([128, S], F32)
                            for ch in range(2):
                                nc.tensor.matmul(
                                    ps_sh[:, ch * 512:(ch + 1) * 512],
                                    _r(permT_sb[:]),
                                    _r(qk_sb[:, ch * 512:(ch + 1) * 512]),
                                    start=True, stop=True,
                                )
                            t1 = rt.tile([128, S], F32, tag="t1")
                            nc.vector.scalar_tensor_tensor(
                                t1[:], ps_qk[:], bqk_sb[:, m:m + 1], cos_sb[:],
                                op0=ALU.add, op1=ALU.mult)
                            s2 = rt.tile([128, S], F32, tag="s2")
                            nc.vector.scalar_tensor_tensor(
                                s2[:], ps_sh[:], bqksh_sb[:, m:m + 1],
                                sin_sb[:], op0=ALU.add, op1=ALU.mult)
                            nc.vector.tensor_add(
                                rope_sb[:, m, :], t1[:], s2[:])

            # ---------------- Phase C: attention per head pair ----------------
            wout_sb = persist.tile([128, 4, HID], F32R)
            for kt in range(4):
                nc.sync.dma_start(
                    wout_sb[:, kt, :], wout[kt * 128:(kt + 1) * 128, :])

            with (
                tc.tile_pool(name="phC", bufs=5) as pc,
                tc.tile_pool(name="psS", bufs=1, space="PSUM") as psS,
                tc.tile_pool(name="psC", bufs=1, space="PSUM") as psC,
                tc.tile_pool(name="small", bufs=4) as small,
                tc.tile_pool(name="drbounce", bufs=2, space="DRAM") as drb,
            ):
                for p in range(4):
                    qp = rope_sb[:, p, :]
                    kp = rope_sb[:, p + 4, :]
                    ps_s0 = psS.tile([128, 2, 512], F32, tag="scores0")
                    ps_s1 = psS.tile([128, 2, 512], F32, tag="scores1")
                    ps_ss = (ps_s0, ps_s1)
                    ps_c0 = psC.tile([65, S], F32, tag="ctx0")
                    ps_c1 = psC.tile([65, S], F32, tag="ctx1")
                    ps_cs = (ps_c0, ps_c1)
                    def ctx_mms(tt, exs):
                        for ch in range(2):
                            for hh in range(2):
                                h = 2 * p + hh
                                nc.tensor.matmul(
                                    ps_cs[hh][:, ch * 512:(ch + 1) * 512],
                                    _r(vmask_sb[:, tt, h * 65:h * 65 + 65]),
                                    _r(exs[ch][:, hh, :]),
                                    start=(tt == 0), stop=(tt == ST - 1),
                                )

                    LAG = 3
                    pending = {}
                    for tt in range(ST):
                        exs = []
                        for ch in range(2):
                            for hh in range(2):
                                base = hh * 64
                                nc.tensor.matmul(
                                    ps_ss[ch][:, hh, :],
                                    _r(kp[base:base + 64,
                                          tt * 128:(tt + 1) * 128]),
                                    _r(qp[base:base + 64,
                                          ch * 512:(ch + 1) * 512]),
                                    start=True, stop=True,
                                    tile_position=(base, 0),
                                )
                            ex = pc.tile([128, 2, 512], F32R,
                                         tag=f"expT{ch}", name=f"ex{ch}")
                            nc.scalar.activation(
                                ex[:], ps_ss[ch][:], AF.Exp, scale=0.125)
                            exs.append(ex)
                        pending[tt] = exs
                        if tt >= LAG:
                            ctx_mms(tt - LAG, pending.pop(tt - LAG))
                    for tt in sorted(pending):
                        ctx_mms(tt, pending[tt])
                    # softmax denominators -> reciprocal -> DMA broadcast
                    rc0 = small.tile([1, S], F32, tag="recip0")
                    rc1 = small.tile([1, S], F32, tag="recip1")
                    nc.vector.reciprocal(rc0[:], ps_c0[64:65, :])
                    nc.vector.reciprocal(rc1[:], ps_c1[64:65, :])
                    bounce = drb.tile([2, S], F32)
                    nc.gpsimd.dma_start(bounce[0:1, :], rc0[:])
                    nc.gpsimd.dma_start(bounce[1:2, :], rc1[:])
                    rb = pc.tile([128, S], F32, tag="rb")
                    nc.gpsimd.dma_start(
                        rb[0:64, :], bounce[0:1, :].broadcast_to([64, S]))
                    nc.gpsimd.dma_start(
                        rb[64:128, :], bounce[1:2, :].broadcast_to([64, S]))
                    nc.vector.tensor_mul(
                        ctxn_sb[0:64, p, :], ps_c0[0:64, :], rb[0:64, :])
                    nc.vector.tensor_mul(
                        ctxn_sb[64:128, p, :], ps_c1[0:64, :], rb[64:128, :])

            # ---------------- Phase D: partial out-projection ----------------
            with (
                tc.tile_pool(name="phD", bufs=3) as pd,
                tc.tile_pool(name="psD", bufs=1, space="PSUM") as psD,
            ):
                for grp in range(2):
                    mg = range(grp * 4, grp * 4 + 4)
                    psy = {m: psD.tile([128, S], F32, tag=f"y{m % 4}",
                                       name=f"psy{m}") for m in mg}
                    for kt in range(3):
                        for m in mg:
                            for ch in range(2):
                                nc.tensor.matmul(
                                    psy[m][:, ch * 512:(ch + 1) * 512],
                                    _r(wout_sb[:, kt, m * 128:(m + 1) * 128]),
                                    _r(ctxn_sb[:, kt, ch * 512:(ch + 1) * 512]),
                                    start=(kt == 0), stop=False,
                                )
                    for m in mg:
                        for ch in range(2):
                            nc.tensor.matmul(
                                psy[m][:, ch * 512:(ch + 1) * 512],
                                _r(wout_sb[:, 3, m * 128:(m + 1) * 128]),
                                _r(ctxn_sb[:, 3, ch * 512:(ch + 1) * 512]),
                                start=False, stop=True,
                            )
                        yt = pd.tile([128, S], F32, tag="yt", name=f"yt{m}")
                        nc.scalar.copy(yt[:], psy[m][:])
                        nc.sync.dma_start(yT[m * 128:(m + 1) * 128, :], yt[:])

    return nc


def _split_waits(nc, max_waits=1):
    """This walrus build rejects >1 sync-wait command per instruction; hoist
    extra waits onto preceding NoOps on the same engine/queue."""
    for bb in nc.main_func.blocks:
        new_insts = []
        for ins in bb.instructions:
            si = getattr(ins, "sync_info", None)
            if si is not None and si.on_wait and len(si.on_wait) > max_waits:
                waits = list(si.on_wait)
                head, rest = waits[:max_waits], waits[max_waits:]
                while rest:
                    chunk, rest = rest[:max_waits], rest[max_waits:]
                    new_insts.append(mybir.InstNoOp(
                        name=f"waitsplit-{nc.next_id()}", ins=[], outs=[],
                        sync_info=mybir.SyncInfo(on_wait=chunk, on_update=[]),
                        engine=ins.engine))
                ins.sync_info = mybir.SyncInfo(
                    on_wait=head, on_update=list(si.on_update or []))
            new_insts.append(ins)
        bb.instructions = new_insts


def make_core_inputs(x, attention_mask, Wqkv, bqkv, Wout):
    """Host-side shard prep: returns list of 8 in_maps (core = 2*b + g)."""
    Wr = np.ascontiguousarray(Wqkv).reshape(HID, 3, H, D)
    br = np.ascontiguousarray(bqkv).reshape(3, H, D)

    inv = 1.0 / (THETA ** (np.arange(0, D, 2, dtype=np.float64) / D))
    pos = np.arange(S, dtype=np.float64)
    freqs = pos[:, None] * inv[None, :]              # [S, 32]
    emb = np.concatenate([freqs, freqs], axis=1)     # [S, 64]
    cosT = np.cos(emb).T.astype(np.float32)          # [64, S]
    sgn = np.concatenate([-np.ones(32), np.ones(32)])[:, None]
    sinTs = (sgn * np.sin(emb).T).astype(np.float32)
    cos2 = np.concatenate([cosT, cosT], 0)           # [128, S]
    sin2 = np.concatenate([sinTs, sinTs], 0)

    in_maps = []
    for c in range(NCORES):
        b, g = c // 2, c % 2
        hs = slice(g * HPC, (g + 1) * HPC)
        wqk = np.concatenate(
            [Wr[:, 0, hs, :].reshape(HID, 512),
             Wr[:, 1, hs, :].reshape(HID, 512)], axis=1)
        wv = Wr[:, 2, hs, :].reshape(HID, 512)
        bqk = np.concatenate(
            [br[0, hs].reshape(512), br[1, hs].reshape(512)]
        ).reshape(8, 128).T
        pp = np.arange(128)
        shmap = (pp - pp % 64) + (pp % 64 + 32) % 64
        bqksh = bqk[shmap]
        permT = np.zeros((128, 128), dtype=np.float32)
        permT[shmap, pp] = 1.0
        mcolv = attention_mask[b].astype(np.float32).reshape(ST, 128).T
        in_maps.append({
            "xT": np.ascontiguousarray(x[b].T.astype(np.float32)),
            "wqk": np.ascontiguousarray(wqk.astype(np.float32)),
            "wv": np.ascontiguousarray(wv.astype(np.float32)),
            "bqk": np.ascontiguousarray(bqk.astype(np.float32)),
            "bqksh": np.ascontiguousarray(bqksh.astype(np.float32)),
            "permT": permT,
            "cosT": cos2, "sinT": sin2,
            "mcol": np.ascontiguousarray(mcolv),
            "wout": np.ascontiguousarray(
                Wout[g * 512:(g + 1) * 512, :].astype(np.float32)),
        })
    return in_maps


_PROGRAM = None


def kernel(x, attention_mask, Wqkv, bqkv, Wout, bout, _trace=False):
    global _PROGRAM
    x = np.asarray(x)
    attention_mask = np.asarray(attention_mask)
    Wqkv = np.asarray(Wqkv)
    bqkv = np.asarray(bqkv)
    Wout = np.asarray(Wout)
    bout = np.asarray(bout)

    if _PROGRAM is None:
        _PROGRAM = build_program()
        _split_waits(_PROGRAM)
    nc = _PROGRAM

    in_maps = make_core_inputs(x, attention_mask, Wqkv, bqkv, Wout)
    res = run_bass_kernel_spmd(
        nc, in_maps, core_ids=list(range(NCORES)), trace=_trace)

    y = np.empty((B, S, HID), dtype=np.float32)
    for b in range(B):
        acc = res.results[2 * b]["yT"] + res.results[2 * b + 1]["yT"]
        y[b] = acc.T
    # exact host-side bias corrections: v-bias shifts context by a constant
    # (attn rows sum to 1), q/k biases were applied on device.
    bv = bqkv[2 * HID:3 * HID].astype(np.float32)
    y += (bv @ Wout + bout).astype(np.float32)[None, None, :]
    if _trace:
        kernel.last_exec_time_ns = res.exec_time_ns
    return y


# revision 19
# speedup vs baseline: 1.0271x; 1.0271x over previous
"""DeBERTa-RoPE self-attention on 8 Trainium2 cores.

Sharding: data-parallel over batch (4) x tensor-parallel over heads (2 groups
of 8). Each core computes qkv projection for its (batch, head-group), RoPE,
attention, and a row-parallel partial out-projection. The host sums the two
partials per batch (the TP all-reduce) and assembles the full output.

Device layout is "transposed world": activations live as [dims, seq] so every
matmul contraction sits on the partition axis with no on-device transposes.
Masking is folded into v (and an appended mask column produces the softmax
denominator for free); softmax skips max-subtraction (|scores| <= ~5 here).
"""

import numpy as np

import concourse.bass as bass
import concourse.mybir as mybir
import concourse.tile as tile
from concourse.bass_utils import run_bass_kernel_spmd

H = 16
D = 64
HID = H * D
B = 4
S = 1024
THETA = 10000.0
NCORES = 8
HPC = H // 2          # heads per core
KT = HID // 128       # 8 k-tiles
ST = S // 128         # 8 seq tiles

F32 = mybir.dt.float32
F32R = mybir.dt.float32r
AF = mybir.ActivationFunctionType
ALU = mybir.AluOpType


def _r(ap):
    return ap.bitcast(F32R)


def build_program():
    nc = bass.Bass()
    xT = nc.declare_dram_parameter("xT", [HID, S], F32R, isOutput=False)
    wqk = nc.declare_dram_parameter("wqk", [HID, 1024], F32R, isOutput=False)
    wv = nc.declare_dram_parameter("wv", [HID, 512], F32R, isOutput=False)
    bqk = nc.declare_dram_parameter("bqk", [128, 8], F32, isOutput=False)
    bqksh = nc.declare_dram_parameter("bqksh", [128, 8], F32, isOutput=False)
    cosT = nc.declare_dram_parameter("cosT", [128, S], F32, isOutput=False)
    sinT = nc.declare_dram_parameter("sinT", [128, S], F32, isOutput=False)
    mcol = nc.declare_dram_parameter("mcol", [128, ST], F32, isOutput=False)
    wout = nc.declare_dram_parameter("wout", [512, HID], F32R, isOutput=False)
    permT = nc.declare_dram_parameter("permT", [128, 128], F32R, isOutput=False)
    yT = nc.declare_dram_parameter("yT", [HID, S], F32, isOutput=True)

    with tile.TileContext(nc) as tc:
        with (
            tc.tile_pool(name="const", bufs=1) as cpool,
            tc.tile_pool(name="persist", bufs=1) as persist,
        ):
            cos_sb = cpool.tile([128, S], F32)
            sin_sb = cpool.tile([128, S], F32)
            mcol_sb = cpool.tile([128, ST], F32)
            bqk_sb = cpool.tile([128, 8], F32)
            bqksh_sb = cpool.tile([128, 8], F32)
            permT_sb = cpool.tile([128, 128], F32R)

            rope_sb = persist.tile([128, 8, S], F32R)
            vmask_sb = persist.tile([128, ST, HPC * 65], F32R)
            ctxn_sb = persist.tile([128, 4, S], F32R)

            # ---------------- Phase A: projections + RoPE + v ----------------
            with tc.tile_pool(name="phA", bufs=1) as pa:
                xT_sb = pa.tile([128, KT, S], F32R)
                wqk_sb = pa.tile([128, KT, 1024], F32R)
                wv_sb = pa.tile([128, KT, 512], F32R)
                for kt in range(KT):
                    nc.sync.dma_start(
                        xT_sb[:, kt, :], xT[kt * 128:(kt + 1) * 128, :])
                    nc.gpsimd.dma_start(
                        wv_sb[:, kt, :], wv[kt * 128:(kt + 1) * 128, :])
                nc.gpsimd.dma_start(mcol_sb[:], mcol[:])
                nc.gpsimd.dma_start(bqk_sb[:], bqk[:])
                nc.gpsimd.dma_start(bqksh_sb[:], bqksh[:])
                nc.gpsimd.dma_start(permT_sb[:], permT[:])
                nc.gpsimd.dma_start(cos_sb[:], cosT[:])
                nc.gpsimd.dma_start(sin_sb[:], sinT[:])
                for kt in range(KT):
                    nc.scalar.dma_start(
                        wqk_sb[:, kt, :], wqk[kt * 128:(kt + 1) * 128, :])

                # v natural [t, d]: kt-outer so MMs stream behind the DMAs;
                # two 4-bank epochs so phase-A qk PSUM can allocate early
                with tc.tile_pool(name="psV", bufs=1, space="PSUM") as psV:
                    for ep in range(2):
                        tts = range(ep * 4, ep * 4 + 4)
                        vps = {tt: psV.tile([128, 512], F32, tag=f"v{tt % 4}",
                                            name=f"vps{tt}")
                               for tt in tts}
                        for kt in range(KT):
                            for tt in tts:
                                nc.tensor.matmul(
                                    vps[tt][:],
                                    _r(xT_sb[:, kt, tt * 128:(tt + 1) * 128]),
                                    _r(wv_sb[:, kt, :]),
                                    start=(kt == 0), stop=(kt == KT - 1),
                                )
                        for tt in tts:
                            vv = vmask_sb[:, tt, :].rearrange(
                                "p (h j) -> p h j", j=65)
                            nc.scalar.activation(
                                vv[:, :, 0:64],
                                vps[tt][:].rearrange("p (h d) -> p h d", d=64),
                                AF.Copy, scale=mcol_sb[:, tt:tt + 1])
                            nc.gpsimd.tensor_copy(
                                vv[:, :, 64:65],
                                mcol_sb[:, tt:tt + 1].broadcast_to(
                                    [128, HPC, 1]))

                # qkT in (q-pair, k-pair) chunks; kt-outer inside each chunk
                with (
                    tc.tile_pool(name="qksb", bufs=3) as qkp,
                    tc.tile_pool(name="ropetmp", bufs=3) as rt,
                    tc.tile_pool(name="psA", bufs=1, space="PSUM") as psA,
                    tc.tile_pool(name="psSh", bufs=2, space="PSUM") as psSh,
                ):
                    for p in range(4):
                        ms = (p, p + 4)
                        pss = {m: psA.tile([128, S], F32, tag=f"qk{m // 4}",
                                           name=f"psqk{m}")
                               for m in ms}
                        for kt in range(KT):
                            for m in ms:
                                for ch in range(2):
                                    nc.tensor.matmul(
                                        pss[m][:, ch * 512:(ch + 1) * 512],
                                        _r(wqk_sb[:, kt, m * 128:(m + 1) * 128]),
                                        _r(xT_sb[:, kt, ch * 512:(ch + 1) * 512]),
                                        start=(kt == 0), stop=(kt == KT - 1),
                                    )
                        for m in ms:
                            ps_qk = pss[m]
                            qk_sb = qkp.tile([128, S], F32R, tag="qksb")
                            nc.scalar.copy(qk_sb[:], ps_qk[:])
                            ps_sh = psSh.tile([128, S], F32)
                            for ch in range(2):
                                nc.tensor.matmul(
                                    ps_sh[:, ch * 512:(ch + 1) * 512],
                                    _r(permT_sb[:]),
                                    _r(qk_sb[:, ch * 512:(ch + 1) * 512]),
                                    start=True, stop=True,
                                )
                            t1 = rt.tile([128, S], F32, tag="t1")
                            nc.vector.scalar_tensor_tensor(
                                t1[:], ps_qk[:], bqk_sb[:, m:m + 1], cos_sb[:],
                                op0=ALU.add, op1=ALU.mult)
                            s2 = rt.tile([128, S], F32, tag="s2")
                            nc.vector.scalar_tensor_tensor(
                                s2[:], ps_sh[:], bqksh_sb[:, m:m + 1],
                                sin_sb[:], op0=ALU.add, op1=ALU.mult)
                            nc.vector.tensor_add(
                                rope_sb[:, m, :], t1[:], s2[:])

            # ---------------- Phase C: attention per head pair ----------------
            wout_sb = persist.tile([128, 4, HID], F32R)
            for kt in range(4):
                nc.sync.dma_start(
                    wout_sb[:, kt, :], wout[kt * 128:(kt + 1) * 128, :])

            with (
                tc.tile_pool(name="phC", bufs=5) as pc,
                tc.tile_pool(name="psS", bufs=1, space="PSUM") as psS,
                tc.tile_pool(name="psC", bufs=1, space="PSUM") as psC,
                tc.tile_pool(name="small", bufs=4) as small,
                tc.tile_pool(name="drbounce", bufs=2, space="DRAM") as drb,
            ):
                for p in range(4):
                    qp = rope_sb[:, p, :]
                    kp = rope_sb[:, p + 4, :]
                    ps_s0 = psS.tile([128, 2, 512], F32, tag="scores0")
                    ps_s1 = psS.tile([128, 2, 512], F32, tag="scores1")
                    ps_ss = (ps_s0, ps_s1)
                    ps_c0 = psC.tile([65, S], F32, tag="ctx0")
                    ps_c1 = psC.tile([65, S], F32, tag="ctx1")
                    ps_cs = (ps_c0, ps_c1)
                    def ctx_mms(tt, exs):
                        for ch in range(2):
                            for hh in range(2):
                                h = 2 * p + hh
                                nc.tensor.matmul(
                                    ps_cs[hh][:, ch * 512:(ch + 1) * 512],
                                    _r(vmask_sb[:, tt, h * 65:h * 65 + 65]),
                                    _r(exs[ch][:, hh, :]),
                                    start=(tt == 0), stop=(tt == ST - 1),
                                )

                    LAG = 3
                    pending = {}
                    for tt in range(ST):
                        exs = []
                        for ch in range(2):
                            for hh in range(2):
                                base = hh * 64
                                nc.tensor.matmul(
                                    ps_ss[ch][:, hh, :],
                                    _r(kp[base:base + 64,
                                          tt * 128:(tt + 1) * 128]),
                                    _r(qp[base:base + 64,
                                          ch * 512:(ch + 1) * 512]),
                                    start=True, stop=True,
                                    tile_position=(base, 0),
                                )
                            ex = pc.tile([128, 2, 512], F32R,
                                         tag=f"expT{ch}", name=f"ex{ch}")
                            nc.scalar.activation(
                                ex[:], ps_ss[ch][:], AF.Exp, scale=0.125)
                            exs.append(ex)
                        pending[tt] = exs
                        if tt >= LAG:
                            ctx_mms(tt - LAG, pending.pop(tt - LAG))
                    for tt in sorted(pending):
                        ctx_mms(tt, pending[tt])
                    # softmax denominators -> reciprocal -> DMA broadcast
                    rc0 = small.tile([1, S], F32, tag="recip0")
                    rc1 = small.tile([1, S], F32, tag="recip1")
                    nc.vector.reciprocal(rc0[:], ps_c0[64:65, :])
                    nc.vector.reciprocal(rc1[:], ps_c1[64:65, :])
                    bounce = drb.tile([2, S], F32)
                    rb = pc.tile([128, S], F32, tag="rb")
                    nc.sync.dma_start(bounce[0:1, :], rc0[:])
                    nc.sync.dma_start(
                        rb[0:64, :], bounce[0:1, :].broadcast_to([64, S]))
                    nc.scalar.dma_start(bounce[1:2, :], rc1[:])
                    nc.scalar.dma_start(
                        rb[64:128, :], bounce[1:2, :].broadcast_to([64, S]))
                    nc.vector.tensor_mul(
                        ctxn_sb[0:64, p, :], ps_c0[0:64, :], rb[0:64, :])
                    nc.vector.tensor_mul(
                        ctxn_sb[64:128, p, :], ps_c1[0:64, :], rb[64:128, :])

            # ---------------- Phase D: partial out-projection ----------------
            with (
                tc.tile_pool(name="phD", bufs=3) as pd,
                tc.tile_pool(name="psD", bufs=1, space="PSUM") as psD,
            ):
                for grp in range(4):
                    mg = range(grp * 2, grp * 2 + 2)
                    psy = {m: psD.tile([128, S], F32, tag=f"y{m % 2}",
                                       name=f"psy{m}") for m in mg}
                    for kt in range(3):
                        for m in mg:
                            for ch in range(2):
                                nc.tensor.matmul(
                                    psy[m][:, ch * 512:(ch + 1) * 512],
                                    _r(wout_sb[:, kt, m * 128:(m + 1) * 128]),
                                    _r(ctxn_sb[:, kt, ch * 512:(ch + 1) * 512]),
                                    start=(kt == 0), stop=False,
                                )
                    for m in mg:
                        for ch in range(2):
                            nc.tensor.matmul(
                                psy[m][:, ch * 512:(ch + 1) * 512],
                                _r(wout_sb[:, 3, m * 128:(m + 1) * 128]),
                                _r(ctxn_sb[:, 3, ch * 512:(ch + 1) * 512]),
                                start=False, stop=True,
                            )
                        yt = pd.tile([128, S], F32, tag="yt", name=f"yt{m}")
                        nc.scalar.copy(yt[:], psy[m][:])
                        dma = nc.sync if m % 2 == 0 else nc.gpsimd
                        dma.dma_start(yT[m * 128:(m + 1) * 128, :], yt[:])

    return nc


def _split_waits(nc, max_waits=1):
    """This walrus build rejects >1 sync-wait command per instruction; hoist
    extra waits onto preceding NoOps on the same engine/queue."""
    for bb in nc.main_func.blocks:
        new_insts = []
        for ins in bb.instructions:
            si = getattr(ins, "sync_info", None)
            if si is not None and si.on_wait and len(si.on_wait) > max_waits:
                waits = list(si.on_wait)
                head, rest = waits[:max_waits], waits[max_waits:]
                while rest:
                    chunk, rest = rest[:max_waits], rest[max_waits:]
                    new_insts.append(mybir.InstNoOp(
                        name=f"waitsplit-{nc.next_id()}", ins=[], outs=[],
                        sync_info=mybir.SyncInfo(on_wait=chunk, on_update=[]),
                        engine=ins.engine))
                ins.sync_info = mybir.SyncInfo(
                    on_wait=head, on_update=list(si.on_update or []))
            new_insts.append(ins)
        bb.instructions = new_insts


def make_core_inputs(x, attention_mask, Wqkv, bqkv, Wout):
    """Host-side shard prep: returns list of 8 in_maps (core = 2*b + g)."""
    Wr = np.ascontiguousarray(Wqkv).reshape(HID, 3, H, D)
    br = np.ascontiguousarray(bqkv).reshape(3, H, D)

    inv = 1.0 / (THETA ** (np.arange(0, D, 2, dtype=np.float64) / D))
    pos = np.arange(S, dtype=np.float64)
    freqs = pos[:, None] * inv[None, :]              # [S, 32]
    emb = np.concatenate([freqs, freqs], axis=1)     # [S, 64]
    cosT = np.cos(emb).T.astype(np.float32)          # [64, S]
    sgn = np.concatenate([-np.ones(32), np.ones(32)])[:, None]
    sinTs = (sgn * np.sin(emb).T).astype(np.float32)
    cos2 = np.concatenate([cosT, cosT], 0)           # [128, S]
    sin2 = np.concatenate([sinTs, sinTs], 0)

    in_maps = []
    for c in range(NCORES):
        b, g = c // 2, c % 2
        hs = slice(g * HPC, (g + 1) * HPC)
        wqk = np.concatenate(
            [Wr[:, 0, hs, :].reshape(HID, 512),
             Wr[:, 1, hs, :].reshape(HID, 512)], axis=1)
        wv = Wr[:, 2, hs, :].reshape(HID, 512)
        bqk = np.concatenate(
            [br[0, hs].reshape(512), br[1, hs].reshape(512)]
        ).reshape(8, 128).T
        pp = np.arange(128)
        shmap = (pp - pp % 64) + (pp % 64 + 32) % 64
        bqksh = bqk[shmap]
        permT = np.zeros((128, 128), dtype=np.float32)
        permT[shmap, pp] = 1.0
        mcolv = attention_mask[b].astype(np.float32).reshape(ST, 128).T
        in_maps.append({
            "xT": np.ascontiguousarray(x[b].T.astype(np.float32)),
            "wqk": np.ascontiguousarray(wqk.astype(np.float32)),
            "wv": np.ascontiguousarray(wv.astype(np.float32)),
            "bqk": np.ascontiguousarray(bqk.astype(np.float32)),
            "bqksh": np.ascontiguousarray(bqksh.astype(np.float32)),
            "permT": permT,
            "cosT": cos2, "sinT": sin2,
            "mcol": np.ascontiguousarray(mcolv),
            "wout": np.ascontiguousarray(
                Wout[g * 512:(g + 1) * 512, :].astype(np.float32)),
        })
    return in_maps


_PROGRAM = None


def kernel(x, attention_mask, Wqkv, bqkv, Wout, bout, _trace=False):
    global _PROGRAM
    x = np.asarray(x)
    attention_mask = np.asarray(attention_mask)
    Wqkv = np.asarray(Wqkv)
    bqkv = np.asarray(bqkv)
    Wout = np.asarray(Wout)
    bout = np.asarray(bout)

    if _PROGRAM is None:
        _PROGRAM = build_program()
        _split_waits(_PROGRAM)
    nc = _PROGRAM

    in_maps = make_core_inputs(x, attention_mask, Wqkv, bqkv, Wout)
    res = run_bass_kernel_spmd(
        nc, in_maps, core_ids=list(range(NCORES)), trace=_trace)

    y = np.empty((B, S, HID), dtype=np.float32)
    for b in range(B):
        acc = res.results[2 * b]["yT"] + res.results[2 * b + 1]["yT"]
        y[b] = acc.T
    # exact host-side bias corrections: v-bias shifts context by a constant
    # (attn rows sum to 1), q/k biases were applied on device.
    bv = bqkv[2 * HID:3 * HID].astype(np.float32)
    y += (bv @ Wout + bout).astype(np.float32)[None, None, :]
    if _trace:
        kernel.last_exec_time_ns = res.exec_time_ns
    return y
